# revision 45
# baseline (speedup 1.0000x reference)
"""Multi-head attention (B=8, N=1024, D=768, H=12) on 8 TRN2 NeuronCores.

Sharding: data-parallel over batch B — one batch element per core, weights
replicated, no collectives.

Per-core layout (everything feature-major so no on-chip transposes):
  x^T [768, 1024] (host-transposed, bf16)
  Q/K feature-major [c, n]: lhsT = w_qkv block, rhs = x^T          -> QK_fm
  V token-major  [n, c]:    lhsT = x^T block,  rhs = w_qkv V cols  -> V_tm
  S^T[k, q] per (head, ktile): lhsT = K_fm [64,128], rhs = Q_fm [64,512]
     (the two heads of a pair run as concurrent row-tiled matmuls:
      partitions 0-63 / 64-127 -> tile_position (0,0)/(64,0))
  P^T = exp(SCALE * S^T) on ACT, bf16 out
  AV^T + softmax denominator in one matmul: lhsT = [V | ones] [128, 65]
     -> psum [65, q]: rows 0-63 = (P V)^T, row 64 = rowsum(P)
  normalize: sums row staged to DRAM, partition-broadcast back via a
     step-0 DMA, r = 1/s on DVE (reciprocal_approx_fast), one multiply
     writes straight into the pair-packed proj input tile
     (cross-partition-base write, 32-aligned bases only).
  proj: lhsT = w_proj block, rhs = A_fm -> out_fm [768, 1024] fp32 + bias
Host gathers out_fm per core and transposes back to [B, 1024, 768].

Emission is software-pipelined: per ktile of pair p's body, AV of pair
p comes first (ready work), then S^T of pair p+1 (its two heads chained
with sync=False ordering edges so consecutive matmuls alternate row
groups and overlap in the PE array), then a QK matmul group of pair
p+2 (mid-loop so its PSUM slots recycle mid-pair). One unified 3-slot
[128,1024] PSUM pool serves QK/V/S^T/proj (6 banks) + a 2-slot AV pool
(2 banks); six sacrificial allocations at each pair's end shift the
slot-reuse rotation so the next pair's first S^T tiles depend on
instantly-completing memsets instead of this pair's final exps. A dummy
16-element exp at kernel start preloads the ACT table under the DMA
window; proj m-blocks 0-2 prefill their first 5 contraction steps in
the last pair's exp slack.
"""

import numpy as np
import ml_dtypes

import concourse.bass as bass
import concourse.tile as tile
from concourse import bacc, mybir

FP32 = mybir.dt.float32
BF16 = mybir.dt.bfloat16

B, N, D = 8, 1024, 768
H, HD = 12, 64
SCALE = float(HD) ** -0.5  # 0.125
CB = D // 128  # 6 contraction blocks of 128
PAIRS = H // 2  # 6 head pairs
KT = N // 128  # 8 key-token tiles
QB = N // 512  # 2 q blocks of 512
NCORES = 8


def build_attention(tc, outs, ins):
    from contextlib import ExitStack

    nc = tc.nc
    xT = ins["xT"]  # [768, 1024] bf16 dram
    wqkv = ins["w_qkv"]  # [768, 2304] bf16 dram
    wproj = ins["w_proj"]  # [768, 768] bf16 dram
    bproj = ins["b_proj"]  # [768] fp32 dram
    out = outs["out"]  # [768, 1024] fp32 dram

    Exp = mybir.ActivationFunctionType.Exp

    with ExitStack() as ctx:
        ec = ctx.enter_context
        sb_x = ec(tc.tile_pool(name="sb_x", bufs=CB))
        sb_wqk0 = ec(tc.tile_pool(name="sb_wqk0", bufs=CB))
        sb_wqkr = ec(tc.tile_pool(name="sb_wqkr", bufs=CB))
        sb_wv = ec(tc.tile_pool(name="sb_wv", bufs=CB))
        sb_wproj = ec(tc.tile_pool(name="sb_wproj", bufs=CB))
        sb_bias = ec(tc.tile_pool(name="sb_bias", bufs=1))
        sb_qk = ec(tc.tile_pool(name="sb_qk", bufs=6))
        sb_v = ec(tc.tile_pool(name="sb_v", bufs=KT))
        sb_pt = ec(tc.tile_pool(name="sb_pt", bufs=32))
        sb_stage = ec(tc.tile_pool(name="sb_stage", bufs=3))
        sb_sbc = ec(tc.tile_pool(name="sb_sbc", bufs=2))
        sb_rbc = ec(tc.tile_pool(name="sb_rbc", bufs=2))
        sb_rrow = ec(tc.tile_pool(name="sb_rrow", bufs=2))
        sb_attn = ec(tc.tile_pool(name="sb_attn", bufs=CB))
        sb_out = ec(tc.tile_pool(name="sb_out", bufs=2))
        ps_big = ec(tc.tile_pool(name="ps_big", bufs=3, space="PSUM"))
        ps_av = ec(tc.tile_pool(name="ps_av", bufs=2, space="PSUM"))
        dram = ec(tc.tile_pool(name="dram", bufs=1, space="DRAM"))

        # ---------- loads ordered by first use ----------
        # x qb0 halves + pair-0 Q/K chunks first so the PE starts early
        # x qb0 + pair-0 Q/K chunks interleaved per c-block so QK(0)'s
        # c-sequential accumulation starts as soon as chunk 0 lands
        x_sb = []
        wqk0_sb = []
        for c in range(CB):
            xt = sb_x.tile([128, N], BF16, name=f"x{c}", tag="x")
            nc.sync.dma_start(xt[:, 0:512], xT[c * 128 : (c + 1) * 128, 0:512])
            x_sb.append(xt)
            wt = sb_wqk0.tile([128, 256], BF16, name=f"wqk0_{c}", tag="wqk0")
            rows = wqkv[c * 128 : (c + 1) * 128, :]
            src = bass.AP(
                tensor=rows.tensor,
                offset=rows.offset,
                ap=[rows.ap[0], [D, 2], [1, 128]],
            )
            nc.sync.dma_start(wt.rearrange("p (a b) -> p a b", a=2), src)
            wqk0_sb.append(wt)
        for c in range(CB):
            nc.sync.dma_start(
                x_sb[c][:, 512:1024], xT[c * 128 : (c + 1) * 128, 512:1024]
            )
        wv_sb = []
        for c in range(CB):
            wt = sb_wv.tile([128, D], BF16, name=f"wv{c}", tag="wv")
            nc.sync.dma_start(wt, wqkv[c * 128 : (c + 1) * 128, 2 * D : 3 * D])
            wv_sb.append(wt)
        wqkr_sb = []
        for c in range(CB):
            wt = sb_wqkr.tile([128, 1280], BF16, name=f"wqkr{c}", tag="wqkr")
            rows = wqkv[c * 128 : (c + 1) * 128, :]
            src = bass.AP(
                tensor=rows.tensor,
                offset=rows.offset + 128,
                ap=[rows.ap[0], [D, 2], [1, 640]],
            )
            nc.sync.dma_start(wt.rearrange("p (a b) -> p a b", a=2), src)
            wqkr_sb.append(wt)
        bias_sb = sb_bias.tile([128, CB], FP32, name="bias")
        nc.sync.dma_start(bias_sb, bproj.rearrange("(a p) -> p a", p=128))
        ones_sb = sb_bias.tile([1, 64], FP32, name="ones", tag="ones")
        nc.vector.memset(ones_sb, 1.0)
        s_dram = dram.tile([H, N], FP32, name="s_dram")
        wp_sb = []
        for c in range(CB):
            wt = sb_wproj.tile([128, D], BF16, name=f"wp{c}", tag="wp")
            nc.sync.dma_start(wt, wproj[c * 128 : (c + 1) * 128, :])
            wp_sb.append(wt)

        def wqk_slice(c, p, which):
            if p == 0:
                return wqk0_sb[c][:, which * 128 : (which + 1) * 128]
            return wqkr_sb[c][:, which * 640 + (p - 1) * 128 : which * 640 + p * 128]

        qk_sb = {}  # (which, pair) -> [128, N] bf16

        def emit_qk_group(p, which, qb):
            if (which, p) not in qk_sb:
                qkt = sb_qk.tile([128, N], BF16, name=f"qk{which}_{p}", tag="qk")
                qk_sb[(which, p)] = qkt
            qkt = qk_sb[(which, p)]
            ps = ps_big.tile(
                [128, 512], FP32, name=f"qkps{which}_{p}_{qb}", tag="ps"
            )
            for c in range(CB):
                nc.tensor.matmul(
                    ps,
                    lhsT=wqk_slice(c, p, which),
                    rhs=x_sb[c][:, qb * 512 : (qb + 1) * 512],
                    start=(c == 0),
                    stop=(c == CB - 1),
                )
            nc.vector.tensor_copy(qkt[:, qb * 512 : (qb + 1) * 512], ps)

        def emit_qk(p):
            for qb in range(QB):
                for which in (0, 1):  # 0 = Q, 1 = K
                    emit_qk_group(p, which, qb)

        pt_tiles = {}  # (pair, j, kt) -> [128, N] bf16
        prev_exp = {}  # pair -> exp instruction of previous ktile's j1
        from concourse.tile import add_dep_helper

        def pt_src(halves):
            # halves are two contiguous views of one [128, N] psum tile
            full = halves[0]
            return bass.AP(
                tensor=full.tensor,
                offset=full.offset,
                ap=[full.ap[0], [1, N]],
            )

        def emit_st_pair(p, kt):
            # Both heads' S^T for this ktile with alternating row groups
            # (partitions 0-63 / 64-127) so consecutive matmuls overlap in
            # the PE array (concurrent row-tiled execution). The first
            # ktile of each pair uses qb-split tiles from the ps_qk pool so
            # it is not chained (via PSUM slot reuse) to the previous
            # pair's final exp — keeps the ACT gapless across pairs.
            q_t, k_t = qk_sb[(0, p)], qk_sb[(1, p)]
            split = False
            sts = []
            for j in (0, 1):
                st = ps_big.tile(
                    [128, N], FP32, name=f"st{2*p+j}_{kt}", tag="ps"
                )
                sts.append([st[:, 0:512], st[:, 512:1024]])
            prev_mm = None
            for qb in range(QB):
                for j in (0, 1):
                    mm = nc.tensor.matmul(
                        sts[j][qb],
                        lhsT=k_t[j * 64 : (j + 1) * 64, kt * 128 : (kt + 1) * 128],
                        rhs=q_t[j * 64 : (j + 1) * 64, qb * 512 : (qb + 1) * 512],
                        start=True,
                        stop=True,
                    )
                    # sync=False ordering chain: forces strict j0/j1
                    # alternation in the static PE order so consecutive
                    # S^T matmuls land on different row groups and overlap
                    # in the array (no runtime semaphore cost)
                    if prev_mm is not None:
                        add_dep_helper(
                            mm.ins,
                            prev_mm.ins,
                            sync=False,
                            reason="alternate row groups for PE overlap",
                        )
                    prev_mm = mm
            last_exp = None
            for j in (0, 1):
                pt = sb_pt.tile([128, N], BF16, name=f"pt{2*p+j}_{kt}", tag="pt")
                if split:
                    for qb in range(QB):
                        last_exp = nc.scalar.activation(
                            pt[:, qb * 512 : (qb + 1) * 512],
                            sts[j][qb],
                            Exp,
                            scale=SCALE,
                        )
                else:
                    last_exp = nc.scalar.activation(
                        pt, pt_src(sts[j]), Exp, scale=SCALE
                    )
                pt_tiles[(p, j, kt)] = pt
            prev_exp[p] = last_exp

        # ---------- prologue: QK(0), then S^T/exp(0) interleaved with V ----
        v_sb = []

        def emit_v(t):
            vt = sb_v.tile([128, H * 65], BF16, name=f"v{t}", tag="v")
            nc.vector.memset(vt, 1.0)
            vtr = vt.rearrange("p (h e) -> p h e", h=H)[:, :, 0:HD]
            for n0, nw in ((0, 512), (512, 256)):
                vps = ps_big.tile([128, nw], FP32, name=f"vps{t}_{n0}", tag="ps")
                for c in range(CB):
                    nc.tensor.matmul(
                        vps,
                        lhsT=x_sb[c][:, t * 128 : (t + 1) * 128],
                        rhs=wv_sb[c][:, n0 : n0 + nw],
                        start=(c == 0),
                        stop=(c == CB - 1),
                    )
                # copy into the 65-strided layout: n0=0 covers heads 0-7,
                # n0=512 covers heads 8-11
                h0, h1 = n0 // HD, (n0 + nw) // HD
                nc.vector.tensor_copy(
                    vtr[:, h0:h1, :],
                    vps.rearrange("p (h e) -> p h e", e=HD),
                )
            v_sb.append(vt)

        warm = sb_bias.tile([128, 256], BF16, name="warm", tag="warm")
        nc.vector.memset(warm, 0.0)
        # trigger the ACT exp table load (~2.7 us) during the initial DMA
        # window instead of serializing it before the first real exp
        nc.scalar.activation(warm[0:1, 0:16], warm[0:1, 16:32], Exp, scale=1.0)

        emit_qk(0)
        for kt in range(KT):
            emit_st_pair(0, kt)
            if kt >= 2:
                emit_v(kt - 2)
        for t in range(KT - 2, KT):
            emit_v(t)
        emit_qk(1)

        # ---------- pipelined pairs ----------
        def emit_av_kt(p, j, av_tiles, kt):
            h = 2 * p + j
            for qb in range(QB):
                nc.tensor.matmul(
                    av_tiles[qb],
                    lhsT=v_sb[kt][:, h * 65 : (h + 1) * 65],
                    rhs=pt_tiles[(p, j, kt)][:, qb * 512 : (qb + 1) * 512],
                    start=(kt == 0),
                    stop=(kt == KT - 1),
                )

        def emit_norm(p, j, stage, at):
            h = 2 * p + j
            nc.sync.dma_start(s_dram[h : h + 1, :], stage[64:65, :])
            sbc = sb_sbc.tile([64, N], FP32, name=f"sbc{h}", tag="sbc")
            src = s_dram[h : h + 1, :]
            bcast = bass.AP(
                tensor=src.tensor, offset=src.offset, ap=[[0, 64]] + src.ap[-1:]
            )
            nc.gpsimd.dma_start(sbc, bcast)
            rbc = sb_rbc.tile([64, N], FP32, name=f"rbc{h}", tag="rbc")
            nc.vector.reciprocal_approx_fast(rbc, sbc)
            nc.vector.tensor_mul(at[j * 64 : (j + 1) * 64, :], stage[0:64, :], rbc)

        def emit_norm_fast(p, j, stage, at):
            # Low-latency variant for the final heads (pre-proj critical
            # path): GpSimd rebase + DVE recip + K=1 fp32 ones-matmul
            # broadcast on the otherwise-idle PE. ~2x lower latency than
            # the DRAM round-trip and keeps the PE warm for proj.
            h = 2 * p + j
            srow = sb_rrow.tile([1, N], FP32, name=f"srow{h}", tag="rrow")
            nc.gpsimd.tensor_copy(srow, stage[64:65, :])
            rrow = sb_rrow.tile([1, N], FP32, name=f"rrow{h}", tag="rrow")
            nc.vector.reciprocal_approx_fast(rrow, srow)
            rps = ps_av.tile([64, 512], FP32, name=f"rps{h}0", tag="av")
            rps1 = ps_av.tile([64, 512], FP32, name=f"rps{h}1", tag="av")
            for qb, rp in enumerate((rps, rps1)):
                nc.tensor.matmul(
                    rp,
                    lhsT=ones_sb,
                    rhs=rrow[:, qb * 512 : (qb + 1) * 512],
                    start=True,
                    stop=True,
                )
                nc.vector.tensor_mul(
                    at[j * 64 : (j + 1) * 64, qb * 512 : (qb + 1) * 512],
                    stage[0:64, qb * 512 : (qb + 1) * 512],
                    rp,
                )

        attn_sb = []

        proj_ps = {}

        def emit_proj_k(mb, c_lo, c_hi):
            if mb not in proj_ps:
                t = ps_big.tile([128, N], FP32, name=f"projps{mb}", tag="ps")
                proj_ps[mb] = [t[:, 0:512], t[:, 512:1024]]
            for qb in range(QB):
                for c in range(c_lo, c_hi):
                    nc.tensor.matmul(
                        proj_ps[mb][qb],
                        lhsT=wp_sb[c][:, mb * 128 : (mb + 1) * 128],
                        rhs=attn_sb[c][:, qb * 512 : (qb + 1) * 512],
                        start=(c == 0),
                        stop=(c == CB - 1),
                    )

        def emit_proj_out(mb):
            ot = sb_out.tile([128, N], FP32, name=f"out{mb}", tag="out")
            for qb in range(QB):
                nc.vector.tensor_scalar_add(
                    ot[:, qb * 512 : (qb + 1) * 512],
                    proj_ps[mb][qb],
                    bias_sb[:, mb : mb + 1],
                )
                nc.sync.dma_start(
                    out[mb * 128 : (mb + 1) * 128, qb * 512 : (qb + 1) * 512],
                    ot[:, qb * 512 : (qb + 1) * 512],
                )

        for p in range(PAIRS):
            at = sb_attn.tile([128, N], BF16, name=f"attn{p}", tag="attn")
            attn_sb.append(at)

            # AV(p) head 0, interleaved with S^T/exp of pair p+1 and the
            # QK matmul groups of pair p+2 (mid-loop so their PSUM slots
            # recycle mid-pair, not at the boundary)
            stage0 = sb_stage.tile([65, N], FP32, name=f"stage{2*p}", tag="stage")
            av0 = [
                ps_av.tile([65, 512], FP32, name=f"av{2*p}_{qb}", tag="av")
                for qb in range(QB)
            ]
            for kt in range(KT):
                emit_av_kt(p, 0, av0, kt)
                if p + 1 < PAIRS:
                    emit_st_pair(p + 1, kt)
                if p + 2 < PAIRS and 2 <= kt <= 5:
                    qb_, which_ = divmod(kt - 2, 2)
                    emit_qk_group(p + 2, which_, qb_)
            if p + 1 < PAIRS:
                # sacrificial ps_big allocations: shift the slot-reuse
                # rotation so the next pair's first S^T tiles depend on
                # instantly-completing memsets instead of this pair's
                # final exps (keeps ACT gapless across the boundary)
                for s in range(6):
                    sac = ps_big.tile([128, 8], FP32, name=f"sac{p}_{s}", tag="ps")
                    nc.vector.memset(sac[0:1, 0:8], 0.0)
            for qb in range(QB):
                nc.vector.tensor_copy(stage0[:, qb * 512 : (qb + 1) * 512], av0[qb])
            last = p == PAIRS - 1
            # AV(p) head 1 (allocations precede the head-0 norm so the
            # fast-norm rps tiles land after them in the ps_av rotation)
            stage1 = sb_stage.tile([65, N], FP32, name=f"stage{2*p+1}", tag="stage")
            av1 = [
                ps_av.tile([65, 512], FP32, name=f"av{2*p+1}_{qb}", tag="av")
                for qb in range(QB)
            ]
            emit_norm(p, 0, stage0, at)
            for kt in range(KT):
                emit_av_kt(p, 1, av1, kt)
            for qb in range(QB):
                nc.vector.tensor_copy(stage1[:, qb * 512 : (qb + 1) * 512], av1[qb])
            if last:
                emit_norm_fast(p, 1, stage1, at)
            else:
                emit_norm(p, 1, stage1, at)



        # ---------- output projection + bias ----------
        # mb0/mb1 prefill their first 5 contraction steps while the last
        # pair's normalizations finish (emitted after AV h1 so the final
        # softmax denominator chain starts as early as possible)
        emit_proj_k(0, 0, CB - 1)
        emit_proj_k(1, 0, CB - 1)
        emit_proj_k(2, 0, CB - 1)
        for mb in (0, 1, 2):
            emit_proj_k(mb, CB - 1, CB)
            emit_proj_out(mb)
        emit_proj_k(3, 0, CB)
        emit_proj_k(4, 0, CB)
        emit_proj_out(3)
        emit_proj_k(5, 0, CB)
        emit_proj_out(4)
        emit_proj_out(5)


def build_nc():
    nc = bacc.Bacc(
        "TRN2", target_bir_lowering=False, debug=False, num_devices=NCORES
    )
    ins = {
        "xT": nc.dram_tensor("xT", [D, N], BF16, kind="ExternalInput").ap(),
        "w_qkv": nc.dram_tensor("w_qkv", [D, 3 * D], BF16, kind="ExternalInput").ap(),
        "w_proj": nc.dram_tensor("w_proj", [D, D], BF16, kind="ExternalInput").ap(),
        "b_proj": nc.dram_tensor("b_proj", [D], FP32, kind="ExternalInput").ap(),
    }
    outs = {"out": nc.dram_tensor("out", [D, N], FP32, kind="ExternalOutput").ap()}
    with tile.TileContext(nc) as tc:
        build_attention(tc, outs, ins)
    nc.compile()
    return nc


def make_in_maps(x, w_qkv, w_proj, b_proj):
    xT = np.ascontiguousarray(
        np.transpose(np.asarray(x, np.float32), (0, 2, 1))
    ).astype(ml_dtypes.bfloat16)
    wq = np.asarray(w_qkv, np.float32).astype(ml_dtypes.bfloat16)
    wp = np.asarray(w_proj, np.float32).astype(ml_dtypes.bfloat16)
    bp = np.ascontiguousarray(np.asarray(b_proj, np.float32))
    return [
        {"xT": np.ascontiguousarray(xT[b]), "w_qkv": wq, "w_proj": wp, "b_proj": bp}
        for b in range(B)
    ]


_BUILT = None


def _get_built():
    global _BUILT
    if _BUILT is None:
        _BUILT = build_nc()
    return _BUILT


def kernel(x, w_qkv, w_proj, b_proj):
    from concourse.bass_utils import run_bass_kernel_spmd

    nc = _get_built()
    in_maps = make_in_maps(x, w_qkv, w_proj, b_proj)
    res = run_bass_kernel_spmd(nc, in_maps, core_ids=list(range(NCORES)))
    return np.stack(
        [np.asarray(res.results[b]["out"], np.float32).T for b in range(B)]
    )


# revision 46
# speedup vs baseline: 1.0072x; 1.0072x over previous
"""Multi-head attention (B=8, N=1024, D=768, H=12) on 8 TRN2 NeuronCores.

Sharding: data-parallel over batch B — one batch element per core, weights
replicated, no collectives.

Per-core layout (everything feature-major so no on-chip transposes):
  x^T [768, 1024] (host-transposed, bf16)
  Q/K feature-major [c, n]: lhsT = w_qkv block, rhs = x^T          -> QK_fm
  V token-major  [n, c]:    lhsT = x^T block,  rhs = w_qkv V cols  -> V_tm
  S^T[k, q] per (head, ktile): lhsT = K_fm [64,128], rhs = Q_fm [64,512]
     (the two heads of a pair run as concurrent row-tiled matmuls:
      partitions 0-63 / 64-127 -> tile_position (0,0)/(64,0))
  P^T = exp(SCALE * S^T) on ACT, bf16 out
  AV^T + softmax denominator in one matmul: lhsT = [V | ones] [128, 65]
     -> psum [65, q]: rows 0-63 = (P V)^T, row 64 = rowsum(P)
  normalize: sums row staged to DRAM, partition-broadcast back via a
     step-0 DMA, r = 1/s on DVE (reciprocal_approx_fast), one multiply
     writes straight into the pair-packed proj input tile
     (cross-partition-base write, 32-aligned bases only).
  proj: lhsT = w_proj block, rhs = A_fm -> out_fm [768, 1024] fp32 + bias
Host gathers out_fm per core and transposes back to [B, 1024, 768].

Emission is software-pipelined: per ktile of pair p's body, AV of pair
p comes first (ready work), then S^T of pair p+1 (its two heads chained
with sync=False ordering edges so consecutive matmuls alternate row
groups and overlap in the PE array), then a QK matmul group of pair
p+2 (mid-loop so its PSUM slots recycle mid-pair). One unified 3-slot
[128,1024] PSUM pool serves QK/V/S^T/proj (6 banks) + a 2-slot AV pool
(2 banks); six sacrificial allocations at each pair's end shift the
slot-reuse rotation so the next pair's first S^T tiles depend on
instantly-completing memsets instead of this pair's final exps. A dummy
16-element exp at kernel start preloads the ACT table under the DMA
window; proj m-blocks 0-2 prefill their first 5 contraction steps in
the last pair's exp slack.
"""

import numpy as np
import ml_dtypes

import concourse.bass as bass
import concourse.tile as tile
from concourse import bacc, mybir

FP32 = mybir.dt.float32
BF16 = mybir.dt.bfloat16

B, N, D = 8, 1024, 768
H, HD = 12, 64
SCALE = float(HD) ** -0.5  # 0.125
CB = D // 128  # 6 contraction blocks of 128
PAIRS = H // 2  # 6 head pairs
KT = N // 128  # 8 key-token tiles
QB = N // 512  # 2 q blocks of 512
NCORES = 8


def build_attention(tc, outs, ins):
    from contextlib import ExitStack

    nc = tc.nc
    xT = ins["xT"]  # [768, 1024] bf16 dram
    wqkv = ins["w_qkv"]  # [768, 2304] bf16 dram
    wproj = ins["w_proj"]  # [768, 768] bf16 dram
    bproj = ins["b_proj"]  # [768] fp32 dram
    out = outs["out"]  # [768, 1024] fp32 dram

    Exp = mybir.ActivationFunctionType.Exp

    with ExitStack() as ctx:
        ec = ctx.enter_context
        sb_x = ec(tc.tile_pool(name="sb_x", bufs=CB))
        sb_wqk0 = ec(tc.tile_pool(name="sb_wqk0", bufs=CB))
        sb_wqkr = ec(tc.tile_pool(name="sb_wqkr", bufs=CB))
        sb_wv = ec(tc.tile_pool(name="sb_wv", bufs=CB))
        sb_wproj = ec(tc.tile_pool(name="sb_wproj", bufs=CB))
        sb_bias = ec(tc.tile_pool(name="sb_bias", bufs=1))
        sb_qk = ec(tc.tile_pool(name="sb_qk", bufs=6))
        sb_v = ec(tc.tile_pool(name="sb_v", bufs=KT))
        sb_pt = ec(tc.tile_pool(name="sb_pt", bufs=32))
        sb_stage = ec(tc.tile_pool(name="sb_stage", bufs=3))
        sb_sbc = ec(tc.tile_pool(name="sb_sbc", bufs=2))
        sb_rbc = ec(tc.tile_pool(name="sb_rbc", bufs=2))
        sb_rrow = ec(tc.tile_pool(name="sb_rrow", bufs=2))
        sb_attn = ec(tc.tile_pool(name="sb_attn", bufs=CB))
        sb_out = ec(tc.tile_pool(name="sb_out", bufs=2))
        ps_big = ec(tc.tile_pool(name="ps_big", bufs=3, space="PSUM"))
        ps_av = ec(tc.tile_pool(name="ps_av", bufs=2, space="PSUM"))
        dram = ec(tc.tile_pool(name="dram", bufs=1, space="DRAM"))

        # ---------- loads ordered by first use ----------
        # x qb0 halves + pair-0 Q/K chunks first so the PE starts early
        # x qb0 + pair-0 Q/K chunks interleaved per c-block so QK(0)'s
        # c-sequential accumulation starts as soon as chunk 0 lands
        x_sb = []
        wqk0_sb = []
        for c in range(CB):
            xt = sb_x.tile([128, N], BF16, name=f"x{c}", tag="x")
            nc.sync.dma_start(xt[:, 0:512], xT[c * 128 : (c + 1) * 128, 0:512])
            x_sb.append(xt)
            wt = sb_wqk0.tile([128, 256], BF16, name=f"wqk0_{c}", tag="wqk0")
            rows = wqkv[c * 128 : (c + 1) * 128, :]
            src = bass.AP(
                tensor=rows.tensor,
                offset=rows.offset,
                ap=[rows.ap[0], [D, 2], [1, 128]],
            )
            nc.sync.dma_start(wt.rearrange("p (a b) -> p a b", a=2), src)
            wqk0_sb.append(wt)
        for c in range(CB):
            nc.sync.dma_start(
                x_sb[c][:, 512:1024], xT[c * 128 : (c + 1) * 128, 512:1024]
            )
        wv_sb = []
        for c in range(CB):
            wt = sb_wv.tile([128, D], BF16, name=f"wv{c}", tag="wv")
            nc.sync.dma_start(wt, wqkv[c * 128 : (c + 1) * 128, 2 * D : 3 * D])
            wv_sb.append(wt)
        wqkr_sb = []
        for c in range(CB):
            wt = sb_wqkr.tile([128, 1280], BF16, name=f"wqkr{c}", tag="wqkr")
            rows = wqkv[c * 128 : (c + 1) * 128, :]
            src = bass.AP(
                tensor=rows.tensor,
                offset=rows.offset + 128,
                ap=[rows.ap[0], [D, 2], [1, 640]],
            )
            nc.sync.dma_start(wt.rearrange("p (a b) -> p a b", a=2), src)
            wqkr_sb.append(wt)
        bias_sb = sb_bias.tile([128, CB], FP32, name="bias")
        nc.sync.dma_start(bias_sb, bproj.rearrange("(a p) -> p a", p=128))
        ones_sb = sb_bias.tile([1, 64], FP32, name="ones", tag="ones")
        nc.vector.memset(ones_sb, 1.0)
        s_dram = dram.tile([H, N], FP32, name="s_dram")
        wp_sb = []
        for c in range(CB):
            wt = sb_wproj.tile([128, D], BF16, name=f"wp{c}", tag="wp")
            nc.sync.dma_start(wt, wproj[c * 128 : (c + 1) * 128, :])
            wp_sb.append(wt)

        def wqk_slice(c, p, which):
            if p == 0:
                return wqk0_sb[c][:, which * 128 : (which + 1) * 128]
            return wqkr_sb[c][:, which * 640 + (p - 1) * 128 : which * 640 + p * 128]

        qk_sb = {}  # (which, pair) -> [128, N] bf16

        def emit_qk_group(p, which, qb):
            if (which, p) not in qk_sb:
                qkt = sb_qk.tile([128, N], BF16, name=f"qk{which}_{p}", tag="qk")
                qk_sb[(which, p)] = qkt
            qkt = qk_sb[(which, p)]
            ps = ps_big.tile(
                [128, 512], FP32, name=f"qkps{which}_{p}_{qb}", tag="ps"
            )
            for c in range(CB):
                nc.tensor.matmul(
                    ps,
                    lhsT=wqk_slice(c, p, which),
                    rhs=x_sb[c][:, qb * 512 : (qb + 1) * 512],
                    start=(c == 0),
                    stop=(c == CB - 1),
                )
            nc.vector.tensor_copy(qkt[:, qb * 512 : (qb + 1) * 512], ps)

        def emit_qk(p):
            for qb in range(QB):
                for which in (0, 1):  # 0 = Q, 1 = K
                    emit_qk_group(p, which, qb)

        pt_tiles = {}  # (pair, j, kt) -> [128, N] bf16
        prev_exp = {}  # pair -> exp instruction of previous ktile's j1
        from concourse.tile import add_dep_helper

        def pt_src(halves):
            # halves are two contiguous views of one [128, N] psum tile
            full = halves[0]
            return bass.AP(
                tensor=full.tensor,
                offset=full.offset,
                ap=[full.ap[0], [1, N]],
            )

        def emit_st_pair(p, kt):
            # Both heads' S^T for this ktile with alternating row groups
            # (partitions 0-63 / 64-127) so consecutive matmuls overlap in
            # the PE array (concurrent row-tiled execution). The first
            # ktile of each pair uses qb-split tiles from the ps_qk pool so
            # it is not chained (via PSUM slot reuse) to the previous
            # pair's final exp — keeps the ACT gapless across pairs.
            q_t, k_t = qk_sb[(0, p)], qk_sb[(1, p)]
            split = False
            sts = []
            for j in (0, 1):
                st = ps_big.tile(
                    [128, N], FP32, name=f"st{2*p+j}_{kt}", tag="ps"
                )
                sts.append([st[:, 0:512], st[:, 512:1024]])
            prev_mm = None
            for qb in range(QB):
                for j in (0, 1):
                    mm = nc.tensor.matmul(
                        sts[j][qb],
                        lhsT=k_t[j * 64 : (j + 1) * 64, kt * 128 : (kt + 1) * 128],
                        rhs=q_t[j * 64 : (j + 1) * 64, qb * 512 : (qb + 1) * 512],
                        start=True,
                        stop=True,
                    )
                    # sync=False ordering chain: forces strict j0/j1
                    # alternation in the static PE order so consecutive
                    # S^T matmuls land on different row groups and overlap
                    # in the array (no runtime semaphore cost)
                    if prev_mm is not None:
                        add_dep_helper(
                            mm.ins,
                            prev_mm.ins,
                            sync=False,
                            reason="alternate row groups for PE overlap",
                        )
                    prev_mm = mm
            last_exp = None
            for j in (0, 1):
                pt = sb_pt.tile([128, N], BF16, name=f"pt{2*p+j}_{kt}", tag="pt")
                if split:
                    for qb in range(QB):
                        last_exp = nc.scalar.activation(
                            pt[:, qb * 512 : (qb + 1) * 512],
                            sts[j][qb],
                            Exp,
                            scale=SCALE,
                        )
                else:
                    last_exp = nc.scalar.activation(
                        pt, pt_src(sts[j]), Exp, scale=SCALE
                    )
                pt_tiles[(p, j, kt)] = pt
            prev_exp[p] = last_exp

        # ---------- prologue: QK(0), then S^T/exp(0) interleaved with V ----
        v_sb = []

        def emit_v(t):
            vt = sb_v.tile([128, H * 65], BF16, name=f"v{t}", tag="v")
            nc.vector.memset(vt, 1.0)
            vtr = vt.rearrange("p (h e) -> p h e", h=H)[:, :, 0:HD]
            for n0, nw in ((0, 512), (512, 256)):
                vps = ps_big.tile([128, nw], FP32, name=f"vps{t}_{n0}", tag="ps")
                for c in range(CB):
                    nc.tensor.matmul(
                        vps,
                        lhsT=x_sb[c][:, t * 128 : (t + 1) * 128],
                        rhs=wv_sb[c][:, n0 : n0 + nw],
                        start=(c == 0),
                        stop=(c == CB - 1),
                    )
                # copy into the 65-strided layout: n0=0 covers heads 0-7,
                # n0=512 covers heads 8-11
                h0, h1 = n0 // HD, (n0 + nw) // HD
                nc.vector.tensor_copy(
                    vtr[:, h0:h1, :],
                    vps.rearrange("p (h e) -> p h e", e=HD),
                )
            v_sb.append(vt)

        warm = sb_bias.tile([128, 256], BF16, name="warm", tag="warm")
        nc.vector.memset(warm, 0.0)
        # trigger the ACT exp table load (~2.7 us) during the initial DMA
        # window instead of serializing it before the first real exp
        nc.scalar.activation(warm[0:1, 0:16], warm[0:1, 16:32], Exp, scale=1.0)

        emit_qk(0)
        for kt in range(KT):
            emit_st_pair(0, kt)
            if kt >= 2:
                emit_v(kt - 2)
        for t in range(KT - 2, KT):
            emit_v(t)
        emit_qk(1)

        # ---------- pipelined pairs ----------
        def emit_av_kt(p, j, av_tiles, kt):
            h = 2 * p + j
            for qb in range(QB):
                nc.tensor.matmul(
                    av_tiles[qb],
                    lhsT=v_sb[kt][:, h * 65 : (h + 1) * 65],
                    rhs=pt_tiles[(p, j, kt)][:, qb * 512 : (qb + 1) * 512],
                    start=(kt == 0),
                    stop=(kt == KT - 1),
                )

        def emit_norm(p, j, stage, at):
            h = 2 * p + j
            nc.sync.dma_start(s_dram[h : h + 1, :], stage[64:65, :])
            sbc = sb_sbc.tile([64, N], FP32, name=f"sbc{h}", tag="sbc")
            src = s_dram[h : h + 1, :]
            bcast = bass.AP(
                tensor=src.tensor, offset=src.offset, ap=[[0, 64]] + src.ap[-1:]
            )
            nc.gpsimd.dma_start(sbc, bcast)
            rbc = sb_rbc.tile([64, N], FP32, name=f"rbc{h}", tag="rbc")
            nc.vector.reciprocal_approx_fast(rbc, sbc)
            nc.vector.tensor_mul(at[j * 64 : (j + 1) * 64, :], stage[0:64, :], rbc)

        def emit_norm_fast(p, j, stage, at):
            # Low-latency variant for the final heads (pre-proj critical
            # path): GpSimd rebase + DVE recip + K=1 fp32 ones-matmul
            # broadcast on the otherwise-idle PE. ~2x lower latency than
            # the DRAM round-trip and keeps the PE warm for proj.
            h = 2 * p + j
            srow = sb_rrow.tile([1, N], FP32, name=f"srow{h}", tag="rrow")
            nc.gpsimd.tensor_copy(srow, stage[64:65, :])
            rrow = sb_rrow.tile([1, N], FP32, name=f"rrow{h}", tag="rrow")
            nc.vector.reciprocal_approx_fast(rrow, srow)
            rps = ps_av.tile([64, 512], FP32, name=f"rps{h}0", tag="av")
            rps1 = ps_av.tile([64, 512], FP32, name=f"rps{h}1", tag="av")
            for qb, rp in enumerate((rps, rps1)):
                nc.tensor.matmul(
                    rp,
                    lhsT=ones_sb,
                    rhs=rrow[:, qb * 512 : (qb + 1) * 512],
                    start=True,
                    stop=True,
                )
                nc.vector.tensor_mul(
                    at[j * 64 : (j + 1) * 64, qb * 512 : (qb + 1) * 512],
                    stage[0:64, qb * 512 : (qb + 1) * 512],
                    rp,
                )

        attn_sb = []

        proj_ps = {}

        def emit_proj_k(mb, c_lo, c_hi):
            if mb not in proj_ps:
                t = ps_big.tile([128, N], FP32, name=f"projps{mb}", tag="ps")
                proj_ps[mb] = [t[:, 0:512], t[:, 512:1024]]
            for qb in range(QB):
                for c in range(c_lo, c_hi):
                    nc.tensor.matmul(
                        proj_ps[mb][qb],
                        lhsT=wp_sb[c][:, mb * 128 : (mb + 1) * 128],
                        rhs=attn_sb[c][:, qb * 512 : (qb + 1) * 512],
                        start=(c == 0),
                        stop=(c == CB - 1),
                    )

        def emit_proj_out(mb):
            # alternate bias-evictions between DVE and the (tail-idle) ACT
            # so the final k5 matmuls aren't gated on one engine's queue
            ot = sb_out.tile([128, N], FP32, name=f"out{mb}", tag="out")
            for qb in range(QB):
                dst = ot[:, qb * 512 : (qb + 1) * 512]
                if (mb + qb) % 2 == 0:
                    nc.vector.tensor_scalar_add(
                        dst, proj_ps[mb][qb], bias_sb[:, mb : mb + 1]
                    )
                else:
                    nc.scalar.add(dst, proj_ps[mb][qb], bias_sb[:, mb : mb + 1])
                nc.sync.dma_start(
                    out[mb * 128 : (mb + 1) * 128, qb * 512 : (qb + 1) * 512],
                    dst,
                )

        for p in range(PAIRS):
            at = sb_attn.tile([128, N], BF16, name=f"attn{p}", tag="attn")
            attn_sb.append(at)

            # AV(p) head 0, interleaved with S^T/exp of pair p+1 and the
            # QK matmul groups of pair p+2 (mid-loop so their PSUM slots
            # recycle mid-pair, not at the boundary)
            stage0 = sb_stage.tile([65, N], FP32, name=f"stage{2*p}", tag="stage")
            av0 = [
                ps_av.tile([65, 512], FP32, name=f"av{2*p}_{qb}", tag="av")
                for qb in range(QB)
            ]
            for kt in range(KT):
                emit_av_kt(p, 0, av0, kt)
                if p + 1 < PAIRS:
                    emit_st_pair(p + 1, kt)
                if p + 2 < PAIRS and 2 <= kt <= 5:
                    qb_, which_ = divmod(kt - 2, 2)
                    emit_qk_group(p + 2, which_, qb_)
            if p + 1 < PAIRS:
                # sacrificial ps_big allocations: shift the slot-reuse
                # rotation so the next pair's first S^T tiles depend on
                # instantly-completing memsets instead of this pair's
                # final exps (keeps ACT gapless across the boundary)
                for s in range(6):
                    sac = ps_big.tile([128, 8], FP32, name=f"sac{p}_{s}", tag="ps")
                    nc.vector.memset(sac[0:1, 0:8], 0.0)
            for qb in range(QB):
                nc.vector.tensor_copy(stage0[:, qb * 512 : (qb + 1) * 512], av0[qb])
            last = p == PAIRS - 1
            # AV(p) head 1 (allocations precede the head-0 norm so the
            # fast-norm rps tiles land after them in the ps_av rotation)
            stage1 = sb_stage.tile([65, N], FP32, name=f"stage{2*p+1}", tag="stage")
            av1 = [
                ps_av.tile([65, 512], FP32, name=f"av{2*p+1}_{qb}", tag="av")
                for qb in range(QB)
            ]
            emit_norm(p, 0, stage0, at)
            for kt in range(KT):
                emit_av_kt(p, 1, av1, kt)
            for qb in range(QB):
                nc.vector.tensor_copy(stage1[:, qb * 512 : (qb + 1) * 512], av1[qb])
            if last:
                emit_norm_fast(p, 1, stage1, at)
            else:
                emit_norm(p, 1, stage1, at)



        # ---------- output projection + bias ----------
        # mb0/mb1 prefill their first 5 contraction steps while the last
        # pair's normalizations finish (emitted after AV h1 so the final
        # softmax denominator chain starts as early as possible)
        emit_proj_k(0, 0, CB - 1)
        emit_proj_k(1, 0, CB - 1)
        emit_proj_k(2, 0, CB - 1)
        for mb in (0, 1, 2):
            emit_proj_k(mb, CB - 1, CB)
            emit_proj_out(mb)
        emit_proj_k(3, 0, CB)
        emit_proj_k(4, 0, CB)
        emit_proj_out(3)
        emit_proj_k(5, 0, CB)
        emit_proj_out(4)
        emit_proj_out(5)


def build_nc():
    nc = bacc.Bacc(
        "TRN2", target_bir_lowering=False, debug=False, num_devices=NCORES
    )
    ins = {
        "xT": nc.dram_tensor("xT", [D, N], BF16, kind="ExternalInput").ap(),
        "w_qkv": nc.dram_tensor("w_qkv", [D, 3 * D], BF16, kind="ExternalInput").ap(),
        "w_proj": nc.dram_tensor("w_proj", [D, D], BF16, kind="ExternalInput").ap(),
        "b_proj": nc.dram_tensor("b_proj", [D], FP32, kind="ExternalInput").ap(),
    }
    outs = {"out": nc.dram_tensor("out", [D, N], FP32, kind="ExternalOutput").ap()}
    with tile.TileContext(nc) as tc:
        build_attention(tc, outs, ins)
    nc.compile()
    return nc


def make_in_maps(x, w_qkv, w_proj, b_proj):
    xT = np.ascontiguousarray(
        np.transpose(np.asarray(x, np.float32), (0, 2, 1))
    ).astype(ml_dtypes.bfloat16)
    wq = np.asarray(w_qkv, np.float32).astype(ml_dtypes.bfloat16)
    wp = np.asarray(w_proj, np.float32).astype(ml_dtypes.bfloat16)
    bp = np.ascontiguousarray(np.asarray(b_proj, np.float32))
    return [
        {"xT": np.ascontiguousarray(xT[b]), "w_qkv": wq, "w_proj": wp, "b_proj": bp}
        for b in range(B)
    ]


_BUILT = None


def _get_built():
    global _BUILT
    if _BUILT is None:
        _BUILT = build_nc()
    return _BUILT


def kernel(x, w_qkv, w_proj, b_proj):
    from concourse.bass_utils import run_bass_kernel_spmd

    nc = _get_built()
    in_maps = make_in_maps(x, w_qkv, w_proj, b_proj)
    res = run_bass_kernel_spmd(nc, in_maps, core_ids=list(range(NCORES)))
    return np.stack(
        [np.asarray(res.results[b]["out"], np.float32).T for b in range(B)]
    )


# revision 47
# speedup vs baseline: 1.0084x; 1.0011x over previous
"""Multi-head attention (B=8, N=1024, D=768, H=12) on 8 TRN2 NeuronCores.

Sharding: data-parallel over batch B — one batch element per core, weights
replicated, no collectives.

Per-core layout (everything feature-major so no on-chip transposes):
  x^T [768, 1024] (host-transposed, bf16)
  Q/K feature-major [c, n]: lhsT = w_qkv block, rhs = x^T          -> QK_fm
  V token-major  [n, c]:    lhsT = x^T block,  rhs = w_qkv V cols  -> V_tm
  S^T[k, q] per (head, ktile): lhsT = K_fm [64,128], rhs = Q_fm [64,512]
     (the two heads of a pair run as concurrent row-tiled matmuls:
      partitions 0-63 / 64-127 -> tile_position (0,0)/(64,0))
  P^T = exp(SCALE * S^T) on ACT, bf16 out
  AV^T + softmax denominator in one matmul: lhsT = [V | ones] [128, 65]
     -> psum [65, q]: rows 0-63 = (P V)^T, row 64 = rowsum(P)
  normalize: sums row staged to DRAM, partition-broadcast back via a
     step-0 DMA, r = 1/s on DVE (reciprocal_approx_fast), one multiply
     writes straight into the pair-packed proj input tile
     (cross-partition-base write, 32-aligned bases only).
  proj: lhsT = w_proj block, rhs = A_fm -> out_fm [768, 1024] fp32 + bias
Host gathers out_fm per core and transposes back to [B, 1024, 768].

Emission is software-pipelined: per ktile of pair p's body, AV of pair
p comes first (ready work), then S^T of pair p+1 (its two heads chained
with sync=False ordering edges so consecutive matmuls alternate row
groups and overlap in the PE array), then a QK matmul group of pair
p+2 (mid-loop so its PSUM slots recycle mid-pair). One unified 3-slot
[128,1024] PSUM pool serves QK/V/S^T/proj (6 banks) + a 2-slot AV pool
(2 banks); six sacrificial allocations at each pair's end shift the
slot-reuse rotation so the next pair's first S^T tiles depend on
instantly-completing memsets instead of this pair's final exps. A dummy
16-element exp at kernel start preloads the ACT table under the DMA
window; proj m-blocks 0-2 prefill their first 5 contraction steps in
the last pair's exp slack.
"""

import numpy as np
import ml_dtypes

import concourse.bass as bass
import concourse.tile as tile
from concourse import bacc, mybir

FP32 = mybir.dt.float32
BF16 = mybir.dt.bfloat16

B, N, D = 8, 1024, 768
H, HD = 12, 64
SCALE = float(HD) ** -0.5  # 0.125
CB = D // 128  # 6 contraction blocks of 128
PAIRS = H // 2  # 6 head pairs
KT = N // 128  # 8 key-token tiles
QB = N // 512  # 2 q blocks of 512
NCORES = 8


def build_attention(tc, outs, ins):
    from contextlib import ExitStack

    nc = tc.nc
    xT = ins["xT"]  # [768, 1024] bf16 dram
    wqkv = ins["w_qkv"]  # [768, 2304] bf16 dram
    wproj = ins["w_proj"]  # [768, 768] bf16 dram
    bproj = ins["b_proj"]  # [768] fp32 dram
    out = outs["out"]  # [768, 1024] fp32 dram

    Exp = mybir.ActivationFunctionType.Exp

    with ExitStack() as ctx:
        ec = ctx.enter_context
        sb_x = ec(tc.tile_pool(name="sb_x", bufs=CB))
        sb_wqk0 = ec(tc.tile_pool(name="sb_wqk0", bufs=CB))
        sb_wqkr = ec(tc.tile_pool(name="sb_wqkr", bufs=CB))
        sb_wv = ec(tc.tile_pool(name="sb_wv", bufs=CB))
        sb_wproj = ec(tc.tile_pool(name="sb_wproj", bufs=CB))
        sb_bias = ec(tc.tile_pool(name="sb_bias", bufs=1))
        sb_qk = ec(tc.tile_pool(name="sb_qk", bufs=6))
        sb_v = ec(tc.tile_pool(name="sb_v", bufs=KT))
        sb_pt = ec(tc.tile_pool(name="sb_pt", bufs=32))
        sb_stage = ec(tc.tile_pool(name="sb_stage", bufs=3))
        sb_sbc = ec(tc.tile_pool(name="sb_sbc", bufs=2))
        sb_rbc = ec(tc.tile_pool(name="sb_rbc", bufs=2))
        sb_rrow = ec(tc.tile_pool(name="sb_rrow", bufs=2))
        sb_attn = ec(tc.tile_pool(name="sb_attn", bufs=CB))
        sb_out = ec(tc.tile_pool(name="sb_out", bufs=3))
        ps_big = ec(tc.tile_pool(name="ps_big", bufs=3, space="PSUM"))
        ps_av = ec(tc.tile_pool(name="ps_av", bufs=2, space="PSUM"))
        dram = ec(tc.tile_pool(name="dram", bufs=1, space="DRAM"))

        # ---------- loads ordered by first use ----------
        # x qb0 halves + pair-0 Q/K chunks first so the PE starts early
        # x qb0 + pair-0 Q/K chunks interleaved per c-block so QK(0)'s
        # c-sequential accumulation starts as soon as chunk 0 lands
        x_sb = []
        wqk0_sb = []
        for c in range(CB):
            xt = sb_x.tile([128, N], BF16, name=f"x{c}", tag="x")
            nc.sync.dma_start(xt[:, 0:512], xT[c * 128 : (c + 1) * 128, 0:512])
            x_sb.append(xt)
            wt = sb_wqk0.tile([128, 256], BF16, name=f"wqk0_{c}", tag="wqk0")
            rows = wqkv[c * 128 : (c + 1) * 128, :]
            src = bass.AP(
                tensor=rows.tensor,
                offset=rows.offset,
                ap=[rows.ap[0], [D, 2], [1, 128]],
            )
            nc.sync.dma_start(wt.rearrange("p (a b) -> p a b", a=2), src)
            wqk0_sb.append(wt)
        for c in range(CB):
            nc.sync.dma_start(
                x_sb[c][:, 512:1024], xT[c * 128 : (c + 1) * 128, 512:1024]
            )
        wv_sb = []
        for c in range(CB):
            wt = sb_wv.tile([128, D], BF16, name=f"wv{c}", tag="wv")
            nc.sync.dma_start(wt, wqkv[c * 128 : (c + 1) * 128, 2 * D : 3 * D])
            wv_sb.append(wt)
        wqkr_sb = []
        for c in range(CB):
            wt = sb_wqkr.tile([128, 1280], BF16, name=f"wqkr{c}", tag="wqkr")
            rows = wqkv[c * 128 : (c + 1) * 128, :]
            src = bass.AP(
                tensor=rows.tensor,
                offset=rows.offset + 128,
                ap=[rows.ap[0], [D, 2], [1, 640]],
            )
            nc.sync.dma_start(wt.rearrange("p (a b) -> p a b", a=2), src)
            wqkr_sb.append(wt)
        bias_sb = sb_bias.tile([128, CB], FP32, name="bias")
        nc.sync.dma_start(bias_sb, bproj.rearrange("(a p) -> p a", p=128))
        ones_sb = sb_bias.tile([1, 64], FP32, name="ones", tag="ones")
        nc.vector.memset(ones_sb, 1.0)
        s_dram = dram.tile([H, N], FP32, name="s_dram")
        wp_sb = []
        for c in range(CB):
            wt = sb_wproj.tile([128, D], BF16, name=f"wp{c}", tag="wp")
            nc.sync.dma_start(wt, wproj[c * 128 : (c + 1) * 128, :])
            wp_sb.append(wt)

        def wqk_slice(c, p, which):
            if p == 0:
                return wqk0_sb[c][:, which * 128 : (which + 1) * 128]
            return wqkr_sb[c][:, which * 640 + (p - 1) * 128 : which * 640 + p * 128]

        qk_sb = {}  # (which, pair) -> [128, N] bf16

        def emit_qk_group(p, which, qb):
            if (which, p) not in qk_sb:
                qkt = sb_qk.tile([128, N], BF16, name=f"qk{which}_{p}", tag="qk")
                qk_sb[(which, p)] = qkt
            qkt = qk_sb[(which, p)]
            ps = ps_big.tile(
                [128, 512], FP32, name=f"qkps{which}_{p}_{qb}", tag="ps"
            )
            for c in range(CB):
                nc.tensor.matmul(
                    ps,
                    lhsT=wqk_slice(c, p, which),
                    rhs=x_sb[c][:, qb * 512 : (qb + 1) * 512],
                    start=(c == 0),
                    stop=(c == CB - 1),
                )
            nc.vector.tensor_copy(qkt[:, qb * 512 : (qb + 1) * 512], ps)

        def emit_qk(p):
            for qb in range(QB):
                for which in (0, 1):  # 0 = Q, 1 = K
                    emit_qk_group(p, which, qb)

        pt_tiles = {}  # (pair, j, kt) -> [128, N] bf16
        prev_exp = {}  # pair -> exp instruction of previous ktile's j1
        from concourse.tile import add_dep_helper

        def pt_src(halves):
            # halves are two contiguous views of one [128, N] psum tile
            full = halves[0]
            return bass.AP(
                tensor=full.tensor,
                offset=full.offset,
                ap=[full.ap[0], [1, N]],
            )

        def emit_st_pair(p, kt):
            # Both heads' S^T for this ktile with alternating row groups
            # (partitions 0-63 / 64-127) so consecutive matmuls overlap in
            # the PE array (concurrent row-tiled execution). The first
            # ktile of each pair uses qb-split tiles from the ps_qk pool so
            # it is not chained (via PSUM slot reuse) to the previous
            # pair's final exp — keeps the ACT gapless across pairs.
            q_t, k_t = qk_sb[(0, p)], qk_sb[(1, p)]
            split = False
            sts = []
            for j in (0, 1):
                st = ps_big.tile(
                    [128, N], FP32, name=f"st{2*p+j}_{kt}", tag="ps"
                )
                sts.append([st[:, 0:512], st[:, 512:1024]])
            prev_mm = None
            for qb in range(QB):
                for j in (0, 1):
                    mm = nc.tensor.matmul(
                        sts[j][qb],
                        lhsT=k_t[j * 64 : (j + 1) * 64, kt * 128 : (kt + 1) * 128],
                        rhs=q_t[j * 64 : (j + 1) * 64, qb * 512 : (qb + 1) * 512],
                        start=True,
                        stop=True,
                    )
                    # sync=False ordering chain: forces strict j0/j1
                    # alternation in the static PE order so consecutive
                    # S^T matmuls land on different row groups and overlap
                    # in the array (no runtime semaphore cost)
                    if prev_mm is not None:
                        add_dep_helper(
                            mm.ins,
                            prev_mm.ins,
                            sync=False,
                            reason="alternate row groups for PE overlap",
                        )
                    prev_mm = mm
            last_exp = None
            for j in (0, 1):
                pt = sb_pt.tile([128, N], BF16, name=f"pt{2*p+j}_{kt}", tag="pt")
                if split:
                    for qb in range(QB):
                        last_exp = nc.scalar.activation(
                            pt[:, qb * 512 : (qb + 1) * 512],
                            sts[j][qb],
                            Exp,
                            scale=SCALE,
                        )
                else:
                    last_exp = nc.scalar.activation(
                        pt, pt_src(sts[j]), Exp, scale=SCALE
                    )
                pt_tiles[(p, j, kt)] = pt
            prev_exp[p] = last_exp

        # ---------- prologue: QK(0), then S^T/exp(0) interleaved with V ----
        v_sb = []

        def emit_v(t):
            vt = sb_v.tile([128, H * 65], BF16, name=f"v{t}", tag="v")
            nc.vector.memset(vt, 1.0)
            vtr = vt.rearrange("p (h e) -> p h e", h=H)[:, :, 0:HD]
            for n0, nw in ((0, 512), (512, 256)):
                vps = ps_big.tile([128, nw], FP32, name=f"vps{t}_{n0}", tag="ps")
                for c in range(CB):
                    nc.tensor.matmul(
                        vps,
                        lhsT=x_sb[c][:, t * 128 : (t + 1) * 128],
                        rhs=wv_sb[c][:, n0 : n0 + nw],
                        start=(c == 0),
                        stop=(c == CB - 1),
                    )
                # copy into the 65-strided layout: n0=0 covers heads 0-7,
                # n0=512 covers heads 8-11
                h0, h1 = n0 // HD, (n0 + nw) // HD
                nc.vector.tensor_copy(
                    vtr[:, h0:h1, :],
                    vps.rearrange("p (h e) -> p h e", e=HD),
                )
            v_sb.append(vt)

        warm = sb_bias.tile([128, 256], BF16, name="warm", tag="warm")
        nc.vector.memset(warm, 0.0)
        # trigger the ACT exp table load (~2.7 us) during the initial DMA
        # window instead of serializing it before the first real exp
        nc.scalar.activation(warm[0:1, 0:16], warm[0:1, 16:32], Exp, scale=1.0)

        emit_qk(0)
        for kt in range(KT):
            emit_st_pair(0, kt)
            if kt >= 2:
                emit_v(kt - 2)
        for t in range(KT - 2, KT):
            emit_v(t)
        emit_qk(1)

        # ---------- pipelined pairs ----------
        def emit_av_kt(p, j, av_tiles, kt):
            h = 2 * p + j
            for qb in range(QB):
                nc.tensor.matmul(
                    av_tiles[qb],
                    lhsT=v_sb[kt][:, h * 65 : (h + 1) * 65],
                    rhs=pt_tiles[(p, j, kt)][:, qb * 512 : (qb + 1) * 512],
                    start=(kt == 0),
                    stop=(kt == KT - 1),
                )

        def emit_norm(p, j, stage, at):
            h = 2 * p + j
            nc.sync.dma_start(s_dram[h : h + 1, :], stage[64:65, :])
            sbc = sb_sbc.tile([64, N], FP32, name=f"sbc{h}", tag="sbc")
            src = s_dram[h : h + 1, :]
            bcast = bass.AP(
                tensor=src.tensor, offset=src.offset, ap=[[0, 64]] + src.ap[-1:]
            )
            nc.gpsimd.dma_start(sbc, bcast)
            rbc = sb_rbc.tile([64, N], FP32, name=f"rbc{h}", tag="rbc")
            nc.vector.reciprocal_approx_fast(rbc, sbc)
            nc.vector.tensor_mul(at[j * 64 : (j + 1) * 64, :], stage[0:64, :], rbc)

        def emit_norm_fast(p, j, stage, at):
            # Low-latency variant for the final heads (pre-proj critical
            # path): GpSimd rebase + DVE recip + K=1 fp32 ones-matmul
            # broadcast on the otherwise-idle PE. ~2x lower latency than
            # the DRAM round-trip and keeps the PE warm for proj.
            h = 2 * p + j
            srow = sb_rrow.tile([1, N], FP32, name=f"srow{h}", tag="rrow")
            nc.gpsimd.tensor_copy(srow, stage[64:65, :])
            rrow = sb_rrow.tile([1, N], FP32, name=f"rrow{h}", tag="rrow")
            nc.vector.reciprocal_approx_fast(rrow, srow)
            rps = ps_av.tile([64, 512], FP32, name=f"rps{h}0", tag="av")
            rps1 = ps_av.tile([64, 512], FP32, name=f"rps{h}1", tag="av")
            for qb, rp in enumerate((rps, rps1)):
                nc.tensor.matmul(
                    rp,
                    lhsT=ones_sb,
                    rhs=rrow[:, qb * 512 : (qb + 1) * 512],
                    start=True,
                    stop=True,
                )
                nc.vector.tensor_mul(
                    at[j * 64 : (j + 1) * 64, qb * 512 : (qb + 1) * 512],
                    stage[0:64, qb * 512 : (qb + 1) * 512],
                    rp,
                )

        attn_sb = []

        proj_ps = {}

        def emit_proj_k(mb, c_lo, c_hi):
            if mb not in proj_ps:
                t = ps_big.tile([128, N], FP32, name=f"projps{mb}", tag="ps")
                proj_ps[mb] = [t[:, 0:512], t[:, 512:1024]]
            for qb in range(QB):
                for c in range(c_lo, c_hi):
                    nc.tensor.matmul(
                        proj_ps[mb][qb],
                        lhsT=wp_sb[c][:, mb * 128 : (mb + 1) * 128],
                        rhs=attn_sb[c][:, qb * 512 : (qb + 1) * 512],
                        start=(c == 0),
                        stop=(c == CB - 1),
                    )

        def emit_proj_out(mb):
            # alternate bias-evictions between DVE and the (tail-idle) ACT
            # so the final k5 matmuls aren't gated on one engine's queue
            ot = sb_out.tile([128, N], FP32, name=f"out{mb}", tag="out")
            for qb in range(QB):
                dst = ot[:, qb * 512 : (qb + 1) * 512]
                if (mb + qb) % 2 == 0:
                    nc.vector.tensor_scalar_add(
                        dst, proj_ps[mb][qb], bias_sb[:, mb : mb + 1]
                    )
                else:
                    nc.scalar.add(dst, proj_ps[mb][qb], bias_sb[:, mb : mb + 1])
                nc.sync.dma_start(
                    out[mb * 128 : (mb + 1) * 128, qb * 512 : (qb + 1) * 512],
                    dst,
                )

        for p in range(PAIRS):
            at = sb_attn.tile([128, N], BF16, name=f"attn{p}", tag="attn")
            attn_sb.append(at)

            # AV(p) head 0, interleaved with S^T/exp of pair p+1 and the
            # QK matmul groups of pair p+2 (mid-loop so their PSUM slots
            # recycle mid-pair, not at the boundary)
            stage0 = sb_stage.tile([65, N], FP32, name=f"stage{2*p}", tag="stage")
            av0 = [
                ps_av.tile([65, 512], FP32, name=f"av{2*p}_{qb}", tag="av")
                for qb in range(QB)
            ]
            for kt in range(KT):
                emit_av_kt(p, 0, av0, kt)
                if p + 1 < PAIRS:
                    emit_st_pair(p + 1, kt)
                if p + 2 < PAIRS and 2 <= kt <= 5:
                    qb_, which_ = divmod(kt - 2, 2)
                    emit_qk_group(p + 2, which_, qb_)
            if p + 1 < PAIRS:
                # sacrificial ps_big allocations: shift the slot-reuse
                # rotation so the next pair's first S^T tiles depend on
                # instantly-completing memsets instead of this pair's
                # final exps (keeps ACT gapless across the boundary)
                for s in range(6):
                    sac = ps_big.tile([128, 8], FP32, name=f"sac{p}_{s}", tag="ps")
                    nc.vector.memset(sac[0:1, 0:8], 0.0)
            for qb in range(QB):
                nc.vector.tensor_copy(stage0[:, qb * 512 : (qb + 1) * 512], av0[qb])
            last = p == PAIRS - 1
            # AV(p) head 1 (allocations precede the head-0 norm so the
            # fast-norm rps tiles land after them in the ps_av rotation)
            stage1 = sb_stage.tile([65, N], FP32, name=f"stage{2*p+1}", tag="stage")
            av1 = [
                ps_av.tile([65, 512], FP32, name=f"av{2*p+1}_{qb}", tag="av")
                for qb in range(QB)
            ]
            emit_norm(p, 0, stage0, at)
            for kt in range(KT):
                emit_av_kt(p, 1, av1, kt)
            for qb in range(QB):
                nc.vector.tensor_copy(stage1[:, qb * 512 : (qb + 1) * 512], av1[qb])
            if last:
                emit_norm_fast(p, 1, stage1, at)
            else:
                emit_norm(p, 1, stage1, at)



        # ---------- output projection + bias ----------
        # mb0/mb1 prefill their first 5 contraction steps while the last
        # pair's normalizations finish (emitted after AV h1 so the final
        # softmax denominator chain starts as early as possible)
        emit_proj_k(0, 0, CB - 1)
        emit_proj_k(1, 0, CB - 1)
        emit_proj_k(2, 0, CB - 1)
        for mb in (0, 1, 2):
            emit_proj_k(mb, CB - 1, CB)
            emit_proj_out(mb)
        emit_proj_k(3, 0, CB)
        emit_proj_k(4, 0, CB)
        emit_proj_out(3)
        emit_proj_k(5, 0, CB)
        emit_proj_out(4)
        emit_proj_out(5)


def build_nc():
    nc = bacc.Bacc(
        "TRN2", target_bir_lowering=False, debug=False, num_devices=NCORES
    )
    ins = {
        "xT": nc.dram_tensor("xT", [D, N], BF16, kind="ExternalInput").ap(),
        "w_qkv": nc.dram_tensor("w_qkv", [D, 3 * D], BF16, kind="ExternalInput").ap(),
        "w_proj": nc.dram_tensor("w_proj", [D, D], BF16, kind="ExternalInput").ap(),
        "b_proj": nc.dram_tensor("b_proj", [D], FP32, kind="ExternalInput").ap(),
    }
    outs = {"out": nc.dram_tensor("out", [D, N], FP32, kind="ExternalOutput").ap()}
    with tile.TileContext(nc) as tc:
        build_attention(tc, outs, ins)
    nc.compile()
    return nc


def make_in_maps(x, w_qkv, w_proj, b_proj):
    xT = np.ascontiguousarray(
        np.transpose(np.asarray(x, np.float32), (0, 2, 1))
    ).astype(ml_dtypes.bfloat16)
    wq = np.asarray(w_qkv, np.float32).astype(ml_dtypes.bfloat16)
    wp = np.asarray(w_proj, np.float32).astype(ml_dtypes.bfloat16)
    bp = np.ascontiguousarray(np.asarray(b_proj, np.float32))
    return [
        {"xT": np.ascontiguousarray(xT[b]), "w_qkv": wq, "w_proj": wp, "b_proj": bp}
        for b in range(B)
    ]


_BUILT = None


def _get_built():
    global _BUILT
    if _BUILT is None:
        _BUILT = build_nc()
    return _BUILT


def kernel(x, w_qkv, w_proj, b_proj):
    from concourse.bass_utils import run_bass_kernel_spmd

    nc = _get_built()
    in_maps = make_in_maps(x, w_qkv, w_proj, b_proj)
    res = run_bass_kernel_spmd(nc, in_maps, core_ids=list(range(NCORES)))
    return np.stack(
        [np.asarray(res.results[b]["out"], np.float32).T for b in range(B)]
    )
